# revision 1
# baseline (speedup 1.0000x reference)
"""Bahdanau attention Trainium2 kernel.

Full-input contract: kernel(**inputs) takes the complete unsharded arrays
(B=8, Td=64, Te=1024, D=256, U=128), shards batch-parallel across 8
NeuronCores (one batch element per core), runs a Bass/Tile kernel via
bass_utils.run_bass_kernel_spmd, and returns the full [8, 64, 256] output.

Per-core algorithm (all shapes hardcoded):
  w_encT [U,Te] = W1.T @ encT + b1   (U on partitions)
  w_decT [U,Td] = W2.T @ decT + b2
  for each (t-block, Te-half, group of GT t's):
      z    [U, GT*HE] = w_encT[:, half] + w_decT[:, t]   (DVE, 2x fp32 mode)
      tanh [U, GT*HE] = tanh(z)                          (ACT, fp16 out)
      score_blk[tl, half] += V . tanh_t                  (PE, banded-C trick)
  per t-block: attn = softmax(score) with 1/sum folded into output scale
  out[block] = (exp(score - max) @ enc) * (1/sum)

The V-contraction uses a banded weight matrix C [128, 2*TB-1] with V stored
in column TB-1: lhsT = C[:, TB-1-tl : 2TB-1-tl] puts V at output partition
tl, so TB accumulating matmuls per (block, half) build score [TB, Te]
directly in PSUM with no per-row evacuation.  Te-halves let the main loop
start once half the encoder is loaded; t-blocks let block 0's softmax /
context overlap block 1's main loop.  bV is dropped entirely (softmax is
shift-invariant).
"""

import os

import numpy as np

import concourse.bass as bass
import concourse.tile as tile
from concourse import bacc
from concourse import mybir
from concourse import bass_utils
from concourse.masks import make_identity

B, TD, TE, D, U = 8, 64, 1024, 256, 128
P = 128
NS = TE // P   # 8 encoder-time chunks
ND = D // P    # 2 feature chunks
F32 = mybir.dt.float32
FP16 = mybir.dt.float16  # tanh in [-1,1]: fp16 mantissa beats bf16, same PE rate
AF = mybir.ActivationFunctionType
AX = mybir.AxisListType
OP = mybir.AluOpType

NB = int(os.environ.get("BAHDANAU_NB", "2"))     # t-blocks
NH = int(os.environ.get("BAHDANAU_NH", "2"))     # Te-halves
GT = int(os.environ.get("BAHDANAU_GT", "8"))     # t's per tanh instruction
ZBUFS = int(os.environ.get("BAHDANAU_ZBUFS", "3"))
THBUFS = int(os.environ.get("BAHDANAU_THBUFS", "3"))

TB = TD // NB    # t's per block
HE = TE // NH    # columns per Te-piece


def _make_pools(ctx, tc: tile.TileContext):
    return dict(
        singles=ctx.enter_context(tc.tile_pool(name="singles", bufs=1)),
        psum_mm=ctx.enter_context(tc.tile_pool(name="psum_mm", bufs=1, space="PSUM")),
        psum_tr=ctx.enter_context(tc.tile_pool(name="psum_tr", bufs=3, space="PSUM")),
        psum_sc=ctx.enter_context(tc.tile_pool(name="psum_sc", bufs=NB, space="PSUM")),
        tanh_pool=ctx.enter_context(tc.tile_pool(name="tanh_pool", bufs=THBUFS)),
        z_pool=ctx.enter_context(tc.tile_pool(name="z_pool", bufs=ZBUFS)),
        small=ctx.enter_context(tc.tile_pool(name="small", bufs=2 * NB)),
    )


def _build_kernel(tc: tile.TileContext, pools: dict, ins: dict, outs: dict):
    nc = tc.nc
    enc, dec, W1, b1, W2, b2, V = (
        ins["enc"], ins["dec"], ins["W1"], ins["b1"], ins["W2"], ins["b2"], ins["V"],
    )
    out = outs["out"]

    singles = pools["singles"]
    psum_mm = pools["psum_mm"]
    psum_tr = pools["psum_tr"]
    psum_sc = pools["psum_sc"]
    tanh_pool = pools["tanh_pool"]
    z_pool = pools["z_pool"]
    small = pools["small"]

    ident = singles.tile([P, P], F32)
    make_identity(nc, ident)

    # ---- loads: interleaved across both HWDGE queues so the first Te-half
    # (enc chunks 0-3) plus W1/dec/W2 land as early as possible ----
    enc_sb = singles.tile([P, NS, D], F32)
    dec_sb = singles.tile([TD, D], F32)
    w1_sb = singles.tile([P, ND, U], F32)
    w2_sb = singles.tile([P, ND, U], F32)
    b1_sb = singles.tile([U, 1], F32)
    b2_sb = singles.tile([U, 1], F32)
    v_sb = singles.tile([U, 1], F32)

    def enc_dma(eng, k):
        eng.dma_start(enc_sb[:, k], enc[k * P:(k + 1) * P, :])

    nc.scalar.dma_start(dec_sb, dec)
    for k in range(ND):
        nc.sync.dma_start(w2_sb[:, k], W2[k * P:(k + 1) * P, :])
    nc.scalar.dma_start(b2_sb, b2)
    enc_dma(nc.sync, 0)
    enc_dma(nc.scalar, 1)
    enc_dma(nc.sync, 2)
    enc_dma(nc.scalar, 3)
    for k in range(ND):
        nc.sync.dma_start(w1_sb[:, k], W1[k * P:(k + 1) * P, :])
    nc.scalar.dma_start(b1_sb, b1)
    nc.sync.dma_start(v_sb, V)
    enc_dma(nc.sync, 4)
    enc_dma(nc.scalar, 5)
    enc_dma(nc.sync, 6)
    enc_dma(nc.scalar, 7)

    # ---- encT + w_encT half 0 first (critical path to the first tanh),
    # decT/w_decT in between, then half 1 ----
    encT = singles.tile([P, ND, TE], F32)
    w_encT = singles.tile([U, TE], F32)
    decT = singles.tile([P, ND, TD], F32)
    w_decT = singles.tile([U, TD], F32)

    def build_enc_cols(c0, c1):
        for k in range(c0 // P, (c1 + P - 1) // P):
            pst = psum_tr.tile([P, ND, P], F32, tag="tr")
            for d in range(ND):
                nc.tensor.transpose(pst[:, d], enc_sb[:, k, d * P:(d + 1) * P], ident)
            nc.vector.tensor_copy(encT[:, :, k * P:(k + 1) * P], pst)
        ps = psum_mm.tile([U, 512], F32, tag="mm")
        for k in range(ND):
            nc.tensor.matmul(ps[:, :c1 - c0], w1_sb[:, k], encT[:, k, c0:c1],
                             start=(k == 0), stop=(k == ND - 1))
        nc.vector.tensor_scalar_add(w_encT[:, c0:c1], ps[:, :c1 - c0], b1_sb)

    def build_enc_half(h):
        for n in range(h * HE // 512, (h + 1) * HE // 512):
            build_enc_cols(n * 512, (n + 1) * 512)

    for d in range(ND):
        pst = psum_tr.tile([P, TD], F32, tag="tr")
        nc.tensor.transpose(pst, dec_sb[:, d * P:(d + 1) * P], ident[:TD, :TD])
        nc.vector.tensor_copy(decT[:, d], pst)
    psd = psum_mm.tile([U, TD], F32, tag="mm")
    for k in range(ND):
        nc.tensor.matmul(psd, w2_sb[:, k], decT[:, k],
                         start=(k == 0), stop=(k == ND - 1))
    nc.vector.tensor_scalar_add(w_decT, psd, b2_sb)

    build_enc_half(0)
    for h in range(1, NH):
        build_enc_half(h)

    # banded V matrix: C[:, TB-1] = V, zeros elsewhere; C[:, TB-1-tl : 2TB-1-tl]
    # puts V at output partition tl.  Built here (not with the loads) so the
    # DVE/Pool streams don't head-of-line block the encT evacuations.
    c_band = singles.tile([U, 2 * TB - 1], FP16)
    nc.vector.memset(c_band, 0.0)
    nc.vector.tensor_copy(c_band[:, TB - 1:TB], v_sb)

    # fp16 copy of enc for the fast context matmul (Pool engine is idle).
    # Column D holds ones so the context matmul also produces sum_s(E) in
    # psum column D, replacing a separate reduce for the softmax denominator.
    enc16 = singles.tile([P, NS, D + 1], FP16)
    for k in range(NS):
        nc.gpsimd.tensor_copy(enc16[:, k, :D], enc_sb[:, k])
    nc.gpsimd.memset(enc16[:, :, D:], 1.0)
    ident16 = singles.tile([P, P], FP16)
    make_identity(nc, ident16)

    # ---- main loop over (t-block, Te-half, group); the epilogue (exp, sum,
    # transpose, context matmul) is also split per Te-half so it pipelines
    # behind the score accumulation instead of serializing at the end ----
    for b in range(NB):
        score_ps = psum_sc.tile([TB, TE], F32, tag="score")
        E = singles.tile([TB, TE], FP16, tag=f"E{b}")
        ET = singles.tile([P, NS, TB], FP16, tag=f"ET{b}")
        ctx_ps = psum_mm.tile([TB, D + 1], F32, tag="mm")
        for h in range(NH):
            for g in range(TB // GT):
                fused = b == 0 and h == 0 and g == 0
                last = b == NB - 1 and h == NH - 1 and g == TB // GT - 1
                th = tanh_pool.tile([U, GT * HE], FP16, tag="tanh")
                if fused:
                    for j in range(GT):
                        t = b * TB + g * GT + j
                        nc.scalar.activation(
                            th[:, j * HE:(j + 1) * HE],
                            w_encT[:, h * HE:(h + 1) * HE], AF.Tanh,
                            bias=w_decT[:, t:t + 1], scale=1.0)
                else:
                    z = z_pool.tile([U, GT * HE], F32, tag="z")
                    for j in range(GT):
                        t = b * TB + g * GT + j
                        nc.vector.tensor_scalar_add(
                            z[:, j * HE:(j + 1) * HE],
                            w_encT[:, h * HE:(h + 1) * HE], w_decT[:, t:t + 1])
                    if last:
                        half = GT * HE // 2
                        nc.scalar.activation(th[:, :half], z[:, :half], AF.Tanh)
                        nc.scalar.activation(th[:, half:], z[:, half:], AF.Tanh)
                    else:
                        nc.scalar.activation(th, z, AF.Tanh)
                for j in range(GT):
                    tl = g * GT + j
                    for n in range(HE // 512):
                        nc.tensor.matmul(
                            score_ps[:, h * HE + n * 512: h * HE + (n + 1) * 512],
                            c_band[:, TB - 1 - tl:2 * TB - 1 - tl],
                            th[:, j * HE + n * 512: j * HE + (n + 1) * 512],
                            start=(tl == 0), stop=(tl == TB - 1))

            # per-half epilogue piece. No max-subtraction: |score| <=
            # sum|V_u| ~ 11 so exp stays comfortably in fp32 range, and
            # softmax is shift-invariant so the result matches the reference.
            nc.scalar.activation(E[:, h * HE:(h + 1) * HE],
                                 score_ps[:, h * HE:(h + 1) * HE], AF.Exp)
            for k in range(h * NS // NH, (h + 1) * NS // NH, 2):
                pst = psum_tr.tile([P, 2, TB], FP16, tag="tr")
                for d in range(2):
                    nc.tensor.transpose(pst[:, d],
                                        E[:, (k + d) * P:(k + d + 1) * P],
                                        ident16[:TB, :TB])
                nc.vector.tensor_copy(ET[:, k:k + 2], pst)
            for k in range(h * NS // NH, (h + 1) * NS // NH):
                nc.tensor.matmul(ctx_ps, ET[:, k], enc16[:, k],
                                 start=(k == 0), stop=(k == NS - 1))

        rsum = small.tile([TB, 1], F32, tag="rsum")
        nc.vector.reciprocal(rsum, ctx_ps[:, D:D + 1])
        out_sb = singles.tile([TB, D], F32, tag=f"out{b}")
        nc.vector.tensor_scalar_mul(out_sb, ctx_ps[:, :D], rsum)
        nc.sync.dma_start(out[b * TB:(b + 1) * TB, :], out_sb)


_CACHE = {}


def _get_nc(reps=1):
    if ("nc", reps) in _CACHE:
        return _CACHE[("nc", reps)]
    nc = bacc.Bacc("TRN2", target_bir_lowering=False, debug=False,
                   enable_asserts=True, num_devices=B)
    ins = {
        "enc": nc.dram_tensor("enc", [TE, D], F32, kind="ExternalInput").ap(),
        "dec": nc.dram_tensor("dec", [TD, D], F32, kind="ExternalInput").ap(),
        "W1": nc.dram_tensor("W1", [D, U], F32, kind="ExternalInput").ap(),
        "b1": nc.dram_tensor("b1", [U, 1], F32, kind="ExternalInput").ap(),
        "W2": nc.dram_tensor("W2", [D, U], F32, kind="ExternalInput").ap(),
        "b2": nc.dram_tensor("b2", [U, 1], F32, kind="ExternalInput").ap(),
        "V": nc.dram_tensor("V", [U, 1], F32, kind="ExternalInput").ap(),
    }
    outs = {"out": nc.dram_tensor("out", [TD, D], F32, kind="ExternalOutput").ap()}
    from contextlib import ExitStack
    with tile.TileContext(nc) as tc:
        with ExitStack() as es:
            pools = _make_pools(es, tc)
            if reps == 1:
                _build_kernel(tc, pools, ins, outs)
            else:
                with tc.For_i(0, reps, 1):
                    _build_kernel(tc, pools, ins, outs)
    nc.compile()
    _CACHE[("nc", reps)] = nc
    return nc


def _in_maps(decoder_output, encoder_output, W1, b1, W2, b2, V):
    f = np.float32
    maps = []
    for b in range(B):
        maps.append({
            "enc": np.ascontiguousarray(encoder_output[b], dtype=f),
            "dec": np.ascontiguousarray(decoder_output[b], dtype=f),
            "W1": np.ascontiguousarray(W1, dtype=f),
            "b1": np.ascontiguousarray(np.asarray(b1, dtype=f).reshape(U, 1)),
            "W2": np.ascontiguousarray(W2, dtype=f),
            "b2": np.ascontiguousarray(np.asarray(b2, dtype=f).reshape(U, 1)),
            "V": np.ascontiguousarray(np.asarray(V, dtype=f).reshape(U, 1)),
        })
    return maps


def run(decoder_output, encoder_output, W1, b1, W2, b2, V, bV=None, *,
        trace=False, **trace_kwargs):
    nc = _get_nc()
    maps = _in_maps(decoder_output, encoder_output, W1, b1, W2, b2, V)
    res = bass_utils.run_bass_kernel_spmd(
        nc, maps, core_ids=list(range(B)), trace=trace, **trace_kwargs)
    out = np.stack([r["out"] for r in res.results], axis=0)
    return out.astype(np.float32), res


def kernel(decoder_output, encoder_output, W1, b1, W2, b2, V, bV=None):
    out, _ = run(decoder_output, encoder_output, W1, b1, W2, b2, V, bV)
    return out

